# revision 15
# baseline (speedup 1.0000x reference)
"""AdaptiveFNOFilter1d Trainium2 kernel.

Per-sample pipeline (8 samples -> 8 NeuronCores, pure data parallel):
  rfft4096 (Cooley-Tukey 64x64 as TensorE matmuls) -> complex block-diag MLP
  (relu, softshrink) -> irfft4096 -> +x residual (on host, in fp32).

All device compute in bf16 (fp32 PSUM accumulation). Mode 2048 dropped
(contributes ~1e-4 relative; validated 6e-4 total rel err vs reference).

Layouts (l = l1 + 64*l2, k = k2 + 64*k1, modes stored for k2-slices s=0..32):
  stage A  : Y[k2ri 66, (l1 64, d 768)] = cA.T @ x[l2 64, (l1 d)]   (row-packed 2x)
  T1 (DRAM): -> Y_T[l1ri 128, (s 33, d 768)]
  stage B  : per (s, d-block 96): X[d96, 128] = Y_T-slice.T @ cB[s]
             X cols per s: [dr 32 | mr 32 | di 32 | mi 32] (mr/mi = modes k2=64-s)
  MLP      : channels on partitions, modes on free dim; ACT fuses bias+relu+evac;
             softshrink = relu(v-lam) - relu(-v-lam) as two ACT passes + DVE sub
  T2 (DRAM): -> Xp[128 rows dr/mr/di/mi, (s 33, d 768)]
  stage B' : per s: G2[l1ri 128, 768] = cD[s].T @ Xp-slice
  T3 (DRAM): -> G2_T[k2ri 66, (l1 64, d 768)]
  stage A' : delta[l2 64, (l1 d)] = cAm.T @ G2_T
"""
import numpy as np
import ml_dtypes

L, G, D, NB, BS = 4096, 64, 768, 8, 96
LAM = 0.01
NS = 33
LD = G * D            # 49152
BF = ml_dtypes.bfloat16

_CACHE = {}


# ---------------------------------------------------------------- matrices
def _build_matrices():
    j = np.arange(G)
    ang = 2 * np.pi * np.outer(j, j) / G
    C64, S64 = np.cos(ang), np.sin(ang)
    cA = np.concatenate([C64[:, :NS] / 64.0, -S64[:, :NS] / 64.0], axis=1)

    l1 = np.arange(G)[:, None]
    k1 = np.arange(32)[None, :]

    def bmat(k2):
        th = 2 * np.pi * (k2 + 64.0 * k1) * l1 / L
        c, s = np.cos(th), np.sin(th)
        M = np.zeros((128, 64))
        M[0:64, 0:32] = c
        M[64:128, 0:32] = s
        M[0:64, 32:64] = -s
        M[64:128, 32:64] = c
        return M

    cB = np.zeros((NS, 128, 128))
    for s in range(NS):
        direct = bmat(s)
        cB[s, :, 0:32] = direct[:, 0:32]
        cB[s, :, 64:96] = direct[:, 32:64]
        if 0 < s < 32:
            mir = bmat(64 - s)
            mir[64:128, :] *= -1.0
            cB[s, :, 32:64] = mir[:, 0:32]
            cB[s, :, 96:128] = mir[:, 32:64]

    l1r = np.arange(G)[None, :]
    k1c = np.arange(32)[:, None]
    cD = np.zeros((NS, 128, 128))
    for s in range(NS):
        th = 2 * np.pi * (s + 64.0 * k1c) * l1r / L
        c, s_ = np.cos(th), np.sin(th)
        dir_r = np.concatenate([c, s_], axis=1)
        dir_i = np.concatenate([-s_, c], axis=1)
        if 0 < s < 32:
            thm = 2 * np.pi * (s + 64.0 * (63 - k1c)) * l1r / L
        elif s == 0:
            thm = 2 * np.pi * (64.0 * ((64 - k1c) % 64)) * l1r / L
        else:
            thm = 2 * np.pi * (32 + 64.0 * (63 - k1c)) * l1r / L
        cm, sm = np.cos(thm), np.sin(thm)
        mir_r = np.concatenate([cm, sm], axis=1)
        mir_i = np.concatenate([sm, -cm], axis=1)
        if s == 0:
            mir_r[0, :] = 0.0
            mir_i[0, :] = 0.0
        if s in (0, 32):
            cD[s, 0:32] = dir_r + mir_r
            cD[s, 64:96] = dir_i + mir_i
        else:
            cD[s, 0:32] = dir_r
            cD[s, 32:64] = mir_r
            cD[s, 64:96] = dir_i
            cD[s, 96:128] = mir_i

    w = np.full(NS, 2.0)
    w[0] = 1.0
    w[32] = 1.0
    cAm = np.concatenate([(w[:, None] * C64[:NS, :]) / 64.0,
                          (w[:, None] * -S64[:NS, :]) / 64.0], axis=0)
    return cA, cB, cD, cAm


# ---------------------------------------------------------------- graph
def _build_graph():
    import concourse.bass as bass
    import concourse.mybir as mybir
    import concourse.tile as tile

    f32 = mybir.dt.float32
    bf16 = mybir.dt.bfloat16
    RELU = mybir.ActivationFunctionType.Relu
    COPY = mybir.ActivationFunctionType.Copy

    nc = bass.Bass()
    x_p = nc.declare_dram_parameter("x", [G, LD], bf16, isOutput=False)
    cA_p = nc.declare_dram_parameter("cA", [G, 66], bf16, isOutput=False)
    cB_p = nc.declare_dram_parameter("cB", [128, NS * 128], bf16, isOutput=False)
    cD_p = nc.declare_dram_parameter("cD", [128, NS * 128], bf16, isOutput=False)
    cAm_p = nc.declare_dram_parameter("cAm", [66, G], bf16, isOutput=False)
    w1_p = nc.declare_dram_parameter("w1s", [BS, NB * 3 * BS], bf16, isOutput=False)
    w2_p = nc.declare_dram_parameter("w2s", [BS, NB * 3 * BS], bf16, isOutput=False)
    b1_p = nc.declare_dram_parameter("b1s", [BS, NB * 2], f32, isOutput=False)
    b2_p = nc.declare_dram_parameter("b2s", [BS, NB * 4], f32, isOutput=False)
    out_p = nc.declare_dram_parameter("delta", [G, LD], bf16, isOutput=True)

    Y_dram = nc.dram_tensor("Y_dram", [66, LD], bf16)
    # o2 spectrum, interleaved per s: cols = (s 33, ri 2, kk 64)
    o2_dram = nc.dram_tensor("o2_dram", [NB, BS, NS * 128], bf16)
    G2_dram = nc.dram_tensor("G2_dram", [128, NS * D], bf16)


    from concourse.tile import add_dep_helper

    def safe_barrier(tc, nc):
        """All-engine barrier that never puts >2 sync waits on one instruction:
        a chain of sync nops each absorbing one producer, then installed as the
        block barrier so every later instruction deps only on the final nop."""
        curr_bb = nc.cur_bb
        prev = list(curr_bb.bb.instructions)
        last_by_engine = {}
        dmas = []
        for i in prev:
            if not i.is_executable():
                continue
            last_by_engine[str(i.engine)] = i
            if "Dma" in type(i).__name__ or "DMA" in type(i).__name__:
                dmas.append(i)
        targets = [v for v in last_by_engine.values()]
        for d in dmas[-8:]:
            if all(d is not t for t in targets):
                targets.append(d)
        n = None
        for t in targets:
            n = nc.sync.nop()
            add_dep_helper(
                n.ins, t,
                sync=bass.sync_unless_reorderable_target(t, t.is_executable()),
                reason="safe_barrier")
        if n is not None:
            tc.barrier_instruction_and_bb = (n.ins, curr_bb)
            if (tc.no_sync_barrier_and_bb is not None
                    and tc.no_sync_barrier_and_bb[1] == curr_bb):
                tc.no_sync_barrier_and_bb = None

    def _split_excess_waits(nc, max_attached=1):
        """Walrus accepts ~1 sync-wait per instruction. Hoist extras onto
        standalone same-engine NoOps inserted immediately before (the raw-bass
        wait_ge idiom), preserving per-engine program order."""
        wid = [0]
        for f in nc.m.functions:
            new_blocks = []
            changed = False
            for b in f.blocks:
                insts = list(b.instructions)
                if not any(i.sync_info and len(i.sync_info.on_wait) > max_attached
                           for i in insts):
                    new_blocks.append(b)
                    continue
                changed = True
                out = []
                for i in insts:
                    si = i.sync_info
                    if si and len(si.on_wait) > max_attached:
                        waits = list(si.on_wait)
                        for w in waits[:-max_attached]:
                            k = mybir.InstNoOp(name=f"I-wsp{wid[0]}", ins=[], outs=[])
                            wid[0] += 1
                            k.engine = i.engine
                            k.sync_info = mybir.SyncInfo(on_wait=[w], on_update=[])
                            out.append(k)
                        i.sync_info = mybir.SyncInfo(
                            on_wait=waits[-max_attached:],
                            on_update=list(si.on_update))
                    out.append(i)
                nb = type(b)(name=b.name, instructions=out)
                nb.IsExit = b.IsExit
                nb.IsLoopEntry = b.IsLoopEntry
                nb.IsPredicated = b.IsPredicated
                new_blocks.append(nb)
            if changed:
                f.blocks = new_blocks

    with tile.TileContext(nc) as tc:
        with tc.tile_pool(name="const", bufs=1) as cpool:
            cA2 = cpool.tile([128, 132], bf16, tag="cA2")
            nc.sync.dma_start(cA2[0:64, 0:66], cA_p[:, :])
            nc.sync.dma_start(cA2[64:128, 66:132], cA_p[:, :])
            cB_s = cpool.tile([128, NS * 128], bf16, tag="cB")
            nc.sync.dma_start(cB_s[:, :], cB_p[:, :])
            cD_s = cpool.tile([128, NS * 128], bf16, tag="cD")
            nc.sync.dma_start(cD_s[:, :], cD_p[:, :])
            cAm_s = cpool.tile([66, G], bf16, tag="cAm")
            nc.sync.dma_start(cAm_s[:, :], cAm_p[:, :])
            w1_s = cpool.tile([BS, NB * 3 * BS], bf16, tag="w1")
            nc.sync.dma_start(w1_s[:, :], w1_p[:, :])
            w2_s = cpool.tile([BS, NB * 3 * BS], bf16, tag="w2")
            nc.sync.dma_start(w2_s[:, :], w2_p[:, :])
            b1_s = cpool.tile([BS, NB * 2], f32, tag="b1")
            nc.sync.dma_start(b1_s[:, :], b1_p[:, :])
            b2_s = cpool.tile([BS, NB * 4], f32, tag="b2")
            nc.sync.dma_start(b2_s[:, :], b2_p[:, :])

            # ---------------- stage A (row-packed 2x) ----------------
            with tc.tile_pool(name="xs", bufs=1) as xpool, \
                 tc.tile_pool(name="ys", bufs=1) as ypool, \
                 tc.tile_pool(name="psA", bufs=2, space="PSUM") as psA:
                x_s = xpool.tile([128, LD // 2], bf16, tag="xs")
                nc.sync.dma_start(x_s[0:64, :], x_p[:, 0:LD // 2])
                nc.sync.dma_start(x_s[64:128, :], x_p[:, LD // 2:LD])
                Y_s = ypool.tile([66, LD], bf16, tag="Ys")
                safe_barrier(tc, nc)
                for c in range(48):
                    sl = slice(512 * c, 512 * (c + 1))
                    sl2 = slice(LD // 2 + 512 * c, LD // 2 + 512 * (c + 1))
                    ps0 = psA.tile([66, 512], f32, tag="ps0")
                    ps1 = psA.tile([66, 512], f32, tag="ps1")
                    nc.tensor.matmul(ps0[:, :], cA2[0:64, 0:66], x_s[0:64, sl],
                                     start=True, stop=True)
                    nc.tensor.matmul(ps1[:, :], cA2[64:128, 66:132], x_s[64:128, sl],
                                     start=True, stop=True)
                    nc.scalar.activation(Y_s[:, sl], ps0[:, :], COPY)
                    nc.vector.tensor_copy(Y_s[:, sl2], ps1[:, :])
                safe_barrier(tc, nc)
                nc.sync.dma_start(Y_dram[:, :], Y_s[:, :])

            # ---------------- T1 + stage B ----------------
            with tc.tile_pool(name="X", bufs=1) as Xpool:
                X_t = [Xpool.tile([BS, NS * 128], bf16, tag=f"X{nb}", name=f"X{nb}")
                       for nb in range(NB)]
                with tc.tile_pool(name="yt", bufs=1) as ytpool, \
                     tc.tile_pool(name="psB", bufs=4, space="PSUM") as psB:
                    Y_T = ytpool.tile([128, NS * D], bf16, tag="YT")
                    nc.sync.dma_start(
                        Y_T[0:64, :].rearrange("p (s d) -> p s d", s=NS),
                        Y_dram[0:NS, :].rearrange("s (l d) -> l s d", l=G))
                    nc.sync.dma_start(
                        Y_T[64:128, :].rearrange("p (s d) -> p s d", s=NS),
                        Y_dram[NS:66, :].rearrange("s (l d) -> l s d", l=G))
                    for g in range(9):
                        ss = list(range(4 * g, min(4 * g + 4, NS)))
                        for nb in range(NB):
                            ps = psB.tile([BS, 512], f32, tag="psB")
                            for si, s in enumerate(ss):
                                nc.tensor.matmul(
                                    ps[:, si * 128:(si + 1) * 128],
                                    Y_T[:, s * D + nb * BS: s * D + nb * BS + BS],
                                    cB_s[:, s * 128:(s + 1) * 128],
                                    start=(si == 0), stop=(si == len(ss) - 1))
                            w_ = len(ss) * 128
                            dst = X_t[nb][:, 512 * g: 512 * g + w_]
                            if (g + nb) % 2 == 0:
                                nc.scalar.activation(dst, ps[:, 0:w_], COPY)
                            else:
                                nc.vector.tensor_copy(dst, ps[:, 0:w_])

                # ---------------- MLP ----------------
                safe_barrier(tc, nc)
                CH = [(0, 8), (8, 8), (16, 8), (24, 8), (32, 1)]  # (s0, n_s)
                with tc.tile_pool(name="o1", bufs=2) as o1pool, \
                     tc.tile_pool(name="o2", bufs=8) as o2pool, \
                     tc.tile_pool(name="sh", bufs=4) as shpool, \
                     tc.tile_pool(name="psM", bufs=2, space="PSUM") as psM:
                    for nb in range(NB):
                        X4 = X_t[nb][:, :].rearrange("p (s h k) -> p s h k", h=2, k=64)
                        w1r = w1_s[:, nb * 288: nb * 288 + 96]
                        w1i = w1_s[:, nb * 288 + 96: nb * 288 + 192]
                        w1ni = w1_s[:, nb * 288 + 192: nb * 288 + 288]
                        o1r = o1pool.tile([BS, NS * 64], bf16, tag="o1r")
                        o1i = o1pool.tile([BS, NS * 64], bf16, tag="o1i")
                        for s0, nsg in CH:
                            n = nsg * 64
                            rr = X4[:, s0:s0 + nsg, 0, :]
                            ri = X4[:, s0:s0 + nsg, 1, :]
                            pr = psM.tile([BS, 512], f32, tag="psMr")
                            pi = psM.tile([BS, 512], f32, tag="psMi")
                            nc.tensor.matmul(pr[:, 0:n], w1r, rr, start=True, stop=False)
                            nc.tensor.matmul(pi[:, 0:n], w1r, ri, start=True, stop=False)
                            nc.tensor.matmul(pr[:, 0:n], w1ni, ri, start=False, stop=True)
                            nc.tensor.matmul(pi[:, 0:n], w1i, rr, start=False, stop=True)
                            nc.scalar.activation(o1r[:, s0 * 64: s0 * 64 + n], pr[:, 0:n],
                                                 RELU, bias=b1_s[:, 2 * nb: 2 * nb + 1])
                            nc.scalar.activation(o1i[:, s0 * 64: s0 * 64 + n], pi[:, 0:n],
                                                 RELU, bias=b1_s[:, 2 * nb + 1: 2 * nb + 2])
                        w2r = w2_s[:, nb * 288: nb * 288 + 96]
                        w2i = w2_s[:, nb * 288 + 96: nb * 288 + 192]
                        w2ni = w2_s[:, nb * 288 + 192: nb * 288 + 288]
                        o2int = o2pool.tile([BS, NS * 128], bf16, tag="o2int")
                        o2v = o2int[:, :].rearrange("p (s x k) -> p s x k", x=2, k=64)
                        for s0, nsg in CH:
                            n = nsg * 64
                            c0 = s0 * 64
                            rr = o1r[:, c0:c0 + n]
                            ri = o1i[:, c0:c0 + n]
                            pr = psM.tile([BS, 512], f32, tag="ps2r")
                            pi = psM.tile([BS, 512], f32, tag="ps2i")
                            nc.tensor.matmul(pr[:, 0:n], w2r, rr, start=True, stop=False)
                            nc.tensor.matmul(pi[:, 0:n], w2r, ri, start=True, stop=False)
                            nc.tensor.matmul(pr[:, 0:n], w2ni, ri, start=False, stop=True)
                            nc.tensor.matmul(pi[:, 0:n], w2i, rr, start=False, stop=True)
                            for (ri_, psrc) in ((0, pr), (1, pi)):
                                bA = b2_s[:, 4 * nb + 2 * ri_: 4 * nb + 2 * ri_ + 1]
                                bC = b2_s[:, 4 * nb + 2 * ri_ + 1: 4 * nb + 2 * ri_ + 2]
                                s1 = shpool.tile([BS, 512], bf16, tag="s1")
                                s2 = shpool.tile([BS, 512], bf16, tag="s2")
                                nc.scalar.activation(s1[:, 0:n], psrc[:, 0:n], RELU,
                                                     bias=bA, scale=1.0)
                                nc.scalar.activation(s2[:, 0:n], psrc[:, 0:n], RELU,
                                                     bias=bC, scale=-1.0)
                                s1v = s1[:, 0:n].rearrange("p (s k) -> p s k", k=64)
                                s2v = s2[:, 0:n].rearrange("p (s k) -> p s k", k=64)
                                nc.vector.tensor_sub(o2v[:, s0:s0 + nsg, ri_, :],
                                                     s1v, s2v)
                        nc.sync.dma_start(o2_dram[nb], o2int[:, :])

            # ---------------- T2 + stage B' ----------------
            safe_barrier(tc, nc)
            with tc.tile_pool(name="g2", bufs=1) as g2pool:
                G2_s = g2pool.tile([128, NS * D], bf16, tag="G2")
                with tc.tile_pool(name="xp", bufs=1) as xppool, \
                     tc.tile_pool(name="psI", bufs=4, space="PSUM") as psI:
                    Xp = xppool.tile([128, NS * D], bf16, tag="Xp")
                    for nb in range(NB):
                        for s in range(NS):
                            nc.sync.dma_start(
                                Xp[:, s * D + nb * BS: s * D + (nb + 1) * BS],
                                o2_dram[nb][:, s * 128:(s + 1) * 128],
                                transpose=True)
                    safe_barrier(tc, nc)
                    for s in range(NS):
                        pa = psI.tile([128, 384], f32, tag="pIa")
                        pb = psI.tile([128, 384], f32, tag="pIb")
                        lhsT = cD_s[:, s * 128:(s + 1) * 128]
                        for nb in range(NB):
                            tgt = pa if nb < 4 else pb
                            col = (nb % 4) * BS
                            nc.tensor.matmul(
                                tgt[:, col:col + BS], lhsT,
                                Xp[:, s * D + nb * BS: s * D + (nb + 1) * BS],
                                start=(nb % 4 == 0), stop=(nb % 4 == 3))
                        nc.scalar.activation(G2_s[:, s * D: s * D + 384], pa[:, :], COPY)
                        nc.vector.tensor_copy(G2_s[:, s * D + 384: s * D + 768], pb[:, :])
                safe_barrier(tc, nc)
                nc.sync.dma_start(G2_dram[:, :], G2_s[:, :])

            # ---------------- T3 + stage A' ----------------
            safe_barrier(tc, nc)
            with tc.tile_pool(name="gt", bufs=1) as gtpool, \
                 tc.tile_pool(name="psO", bufs=2, space="PSUM") as psO, \
                 tc.tile_pool(name="stO", bufs=3) as stO:
                G2_T = gtpool.tile([66, LD], bf16, tag="GT")
                nc.sync.dma_start(
                    G2_T[0:NS, :].rearrange("p (l d) -> p l d", l=G),
                    G2_dram[0:64, :].rearrange("l (s d) -> s l d", s=NS))
                nc.sync.dma_start(
                    G2_T[NS:66, :].rearrange("p (l d) -> p l d", l=G),
                    G2_dram[64:128, :].rearrange("l (s d) -> s l d", s=NS))
                for c in range(96):
                    sl = slice(512 * c, 512 * (c + 1))
                    ps = psO.tile([G, 512], f32, tag="psO")
                    nc.tensor.matmul(ps[:, :], cAm_s[:, :], G2_T[:, sl],
                                     start=True, stop=True)
                    st = stO.tile([G, 512], bf16, tag="stO")
                    if c % 2 == 0:
                        nc.scalar.activation(st[:, :], ps[:, :], COPY)
                    else:
                        nc.vector.tensor_copy(st[:, :], ps[:, :])
                    nc.sync.dma_start(out_p[:, sl], st[:, :])
    _split_excess_waits(nc)
    return nc


def _get_graph():
    if "nc" not in _CACHE:
        _CACHE["nc"] = _build_graph()
    return _CACHE["nc"]


# ---------------------------------------------------------------- host entry
def kernel(x, w1, b1, w2, b2):
    nc = _get_graph()
    from concourse.bass_utils import run_bass_kernel_spmd

    cA, cB, cD, cAm = _build_matrices()
    cB_h = np.ascontiguousarray(cB.transpose(1, 0, 2)).reshape(128, NS * 128)
    cD_h = np.ascontiguousarray(cD.transpose(1, 0, 2)).reshape(128, NS * 128)
    w1_h = np.concatenate(
        [np.concatenate([w1[0, nb], w1[1, nb], -w1[1, nb]], axis=1) for nb in range(NB)],
        axis=1)                                            # [96, 8*288] (rows=i, cols=(nb,t,o))
    w2_h = np.concatenate(
        [np.concatenate([w2[0, nb], w2[1, nb], -w2[1, nb]], axis=1) for nb in range(NB)],
        axis=1)
    b1_h = np.stack([b1[ri, nb] for nb in range(NB) for ri in range(2)], axis=1)  # [96, 16]
    b2_h = np.stack(
        [v for nb in range(NB) for ri in range(2)
         for v in (b2[ri, nb] - LAM, -b2[ri, nb] - LAM)], axis=1)                 # [96, 32]

    consts = {
        "cA": cA.astype(BF), "cB": cB_h.astype(BF), "cD": cD_h.astype(BF),
        "cAm": cAm.astype(BF), "w1s": w1_h.astype(BF), "w2s": w2_h.astype(BF),
        "b1s": b1_h.astype(np.float32), "b2s": b2_h.astype(np.float32),
    }
    B = x.shape[0]
    in_maps = [dict(consts, x=np.ascontiguousarray(x[b].reshape(G, LD)).astype(BF))
               for b in range(B)]
    res = run_bass_kernel_spmd(nc, in_maps, core_ids=list(range(B)))
    _CACHE["last_result"] = res
    _CACHE["last_in_maps"] = in_maps
    y = np.empty_like(x)
    for b in range(B):
        delta = np.asarray(res.results[b]["delta"]).astype(np.float32).reshape(L, D)
        y[b] = x[b] + delta
    return y


# revision 18
# speedup vs baseline: 1.0006x; 1.0006x over previous
"""AdaptiveFNOFilter1d Trainium2 kernel.

Per-sample pipeline (8 samples -> 8 NeuronCores, pure data parallel):
  rfft4096 (Cooley-Tukey 64x64 as TensorE matmuls) -> complex block-diag MLP
  (relu, softshrink) -> irfft4096 -> +x residual (on host, in fp32).

All device compute in bf16 (fp32 PSUM accumulation). Mode 2048 dropped
(contributes ~1e-4 relative; validated 6e-4 total rel err vs reference).

Layouts (l = l1 + 64*l2, k = k2 + 64*k1, modes stored for k2-slices s=0..32):
  stage A  : Y[k2ri 66, (l1 64, d 768)] = cA.T @ x[l2 64, (l1 d)]   (row-packed 2x)
  T1 (DRAM): -> Y_T[l1ri 128, (s 33, d 768)]
  stage B  : per (s, d-block 96): X[d96, 128] = Y_T-slice.T @ cB[s]
             X cols per s: [dr 32 | mr 32 | di 32 | mi 32] (mr/mi = modes k2=64-s)
  MLP      : channels on partitions, modes on free dim; ACT fuses bias+relu+evac;
             softshrink = relu(v-lam) - relu(-v-lam) as two ACT passes + DVE sub
  T2 (DRAM): -> Xp[128 rows dr/mr/di/mi, (s 33, d 768)]
  stage B' : per s: G2[l1ri 128, 768] = cD[s].T @ Xp-slice
  T3 (DRAM): -> G2_T[k2ri 66, (l1 64, d 768)]
  stage A' : delta[l2 64, (l1 d)] = cAm.T @ G2_T
"""
import numpy as np
import ml_dtypes

L, G, D, NB, BS = 4096, 64, 768, 8, 96
LAM = 0.01
NS = 33
LD = G * D            # 49152
BF = ml_dtypes.bfloat16

_CACHE = {}


# ---------------------------------------------------------------- matrices
def _build_matrices():
    j = np.arange(G)
    ang = 2 * np.pi * np.outer(j, j) / G
    C64, S64 = np.cos(ang), np.sin(ang)
    cA = np.concatenate([C64[:, :NS] / 64.0, -S64[:, :NS] / 64.0], axis=1)

    l1 = np.arange(G)[:, None]
    k1 = np.arange(32)[None, :]

    def bmat(k2):
        th = 2 * np.pi * (k2 + 64.0 * k1) * l1 / L
        c, s = np.cos(th), np.sin(th)
        M = np.zeros((128, 64))
        M[0:64, 0:32] = c
        M[64:128, 0:32] = s
        M[0:64, 32:64] = -s
        M[64:128, 32:64] = c
        return M

    cB = np.zeros((NS, 128, 128))
    for s in range(NS):
        direct = bmat(s)
        cB[s, :, 0:32] = direct[:, 0:32]
        cB[s, :, 64:96] = direct[:, 32:64]
        if 0 < s < 32:
            mir = bmat(64 - s)
            mir[64:128, :] *= -1.0
            cB[s, :, 32:64] = mir[:, 0:32]
            cB[s, :, 96:128] = mir[:, 32:64]

    l1r = np.arange(G)[None, :]
    k1c = np.arange(32)[:, None]
    cD = np.zeros((NS, 128, 128))
    for s in range(NS):
        th = 2 * np.pi * (s + 64.0 * k1c) * l1r / L
        c, s_ = np.cos(th), np.sin(th)
        dir_r = np.concatenate([c, s_], axis=1)
        dir_i = np.concatenate([-s_, c], axis=1)
        if 0 < s < 32:
            thm = 2 * np.pi * (s + 64.0 * (63 - k1c)) * l1r / L
        elif s == 0:
            thm = 2 * np.pi * (64.0 * ((64 - k1c) % 64)) * l1r / L
        else:
            thm = 2 * np.pi * (32 + 64.0 * (63 - k1c)) * l1r / L
        cm, sm = np.cos(thm), np.sin(thm)
        mir_r = np.concatenate([cm, sm], axis=1)
        mir_i = np.concatenate([sm, -cm], axis=1)
        if s == 0:
            mir_r[0, :] = 0.0
            mir_i[0, :] = 0.0
        if s in (0, 32):
            cD[s, 0:32] = dir_r + mir_r
            cD[s, 64:96] = dir_i + mir_i
        else:
            cD[s, 0:32] = dir_r
            cD[s, 32:64] = mir_r
            cD[s, 64:96] = dir_i
            cD[s, 96:128] = mir_i

    w = np.full(NS, 2.0)
    w[0] = 1.0
    w[32] = 1.0
    cAm = np.concatenate([(w[:, None] * C64[:NS, :]) / 64.0,
                          (w[:, None] * -S64[:NS, :]) / 64.0], axis=0)
    return cA, cB, cD, cAm


# ---------------------------------------------------------------- graph
def _build_graph():
    import concourse.bass as bass
    import concourse.mybir as mybir
    import concourse.tile as tile

    f32 = mybir.dt.float32
    bf16 = mybir.dt.bfloat16
    RELU = mybir.ActivationFunctionType.Relu
    COPY = mybir.ActivationFunctionType.Copy

    nc = bass.Bass()
    x_p = nc.declare_dram_parameter("x", [G, LD], bf16, isOutput=False)
    cA_p = nc.declare_dram_parameter("cA", [G, 66], bf16, isOutput=False)
    cB_p = nc.declare_dram_parameter("cB", [128, NS * 128], bf16, isOutput=False)
    cD_p = nc.declare_dram_parameter("cD", [128, NS * 128], bf16, isOutput=False)
    cAm_p = nc.declare_dram_parameter("cAm", [66, G], bf16, isOutput=False)
    w1_p = nc.declare_dram_parameter("w1s", [BS, NB * 3 * BS], bf16, isOutput=False)
    w2_p = nc.declare_dram_parameter("w2s", [BS, NB * 3 * BS], bf16, isOutput=False)
    b1_p = nc.declare_dram_parameter("b1s", [BS, NB * 2], f32, isOutput=False)
    b2_p = nc.declare_dram_parameter("b2s", [BS, NB * 4], f32, isOutput=False)
    out_p = nc.declare_dram_parameter("delta", [G, LD], bf16, isOutput=True)

    Y_dram = nc.dram_tensor("Y_dram", [66, LD], bf16)
    # o2 spectrum, interleaved per s: cols = (s 33, ri 2, kk 64)
    o2_dram = nc.dram_tensor("o2_dram", [NB, BS, NS * 128], bf16)


    from concourse.tile import add_dep_helper

    def safe_barrier(tc, nc):
        """All-engine barrier that never puts >2 sync waits on one instruction:
        a chain of sync nops each absorbing one producer, then installed as the
        block barrier so every later instruction deps only on the final nop."""
        curr_bb = nc.cur_bb
        prev = list(curr_bb.bb.instructions)
        last_by_engine = {}
        dmas = []
        for i in prev:
            if not i.is_executable():
                continue
            last_by_engine[str(i.engine)] = i
            if "Dma" in type(i).__name__ or "DMA" in type(i).__name__:
                dmas.append(i)
        targets = [v for v in last_by_engine.values()]
        for d in dmas[-8:]:
            if all(d is not t for t in targets):
                targets.append(d)
        n = None
        for t in targets:
            n = nc.sync.nop()
            add_dep_helper(
                n.ins, t,
                sync=bass.sync_unless_reorderable_target(t, t.is_executable()),
                reason="safe_barrier")
        if n is not None:
            tc.barrier_instruction_and_bb = (n.ins, curr_bb)
            if (tc.no_sync_barrier_and_bb is not None
                    and tc.no_sync_barrier_and_bb[1] == curr_bb):
                tc.no_sync_barrier_and_bb = None

    def _split_excess_waits(nc, max_attached=1):
        """Walrus accepts ~1 sync-wait per instruction. Hoist extras onto
        standalone same-engine NoOps inserted immediately before (the raw-bass
        wait_ge idiom), preserving per-engine program order."""
        wid = [0]
        for f in nc.m.functions:
            new_blocks = []
            changed = False
            for b in f.blocks:
                insts = list(b.instructions)
                if not any(i.sync_info and len(i.sync_info.on_wait) > max_attached
                           for i in insts):
                    new_blocks.append(b)
                    continue
                changed = True
                out = []
                for i in insts:
                    si = i.sync_info
                    if si and len(si.on_wait) > max_attached:
                        waits = list(si.on_wait)
                        for w in waits[:-max_attached]:
                            k = mybir.InstNoOp(name=f"I-wsp{wid[0]}", ins=[], outs=[])
                            wid[0] += 1
                            k.engine = i.engine
                            k.sync_info = mybir.SyncInfo(on_wait=[w], on_update=[])
                            out.append(k)
                        i.sync_info = mybir.SyncInfo(
                            on_wait=waits[-max_attached:],
                            on_update=list(si.on_update))
                    out.append(i)
                nb = type(b)(name=b.name, instructions=out)
                nb.IsExit = b.IsExit
                nb.IsLoopEntry = b.IsLoopEntry
                nb.IsPredicated = b.IsPredicated
                new_blocks.append(nb)
            if changed:
                f.blocks = new_blocks

    with tile.TileContext(nc) as tc:
        with tc.tile_pool(name="const", bufs=1) as cpool:
            cA2 = cpool.tile([128, 132], bf16, tag="cA2")
            nc.sync.dma_start(cA2[0:64, 0:66], cA_p[:, :])
            nc.sync.dma_start(cA2[64:128, 66:132], cA_p[:, :])
            cB_s = cpool.tile([128, NS * 128], bf16, tag="cB")
            nc.sync.dma_start(cB_s[:, :], cB_p[:, :])
            cD_s = cpool.tile([128, NS * 128], bf16, tag="cD")
            nc.sync.dma_start(cD_s[:, :], cD_p[:, :])
            cAm_s = cpool.tile([66, G], bf16, tag="cAm")
            nc.sync.dma_start(cAm_s[:, :], cAm_p[:, :])
            w1_s = cpool.tile([BS, NB * 3 * BS], bf16, tag="w1")
            nc.sync.dma_start(w1_s[:, :], w1_p[:, :])
            w2_s = cpool.tile([BS, NB * 3 * BS], bf16, tag="w2")
            nc.sync.dma_start(w2_s[:, :], w2_p[:, :])
            b1_s = cpool.tile([BS, NB * 2], f32, tag="b1")
            nc.sync.dma_start(b1_s[:, :], b1_p[:, :])
            b2_s = cpool.tile([BS, NB * 4], f32, tag="b2")
            nc.sync.dma_start(b2_s[:, :], b2_p[:, :])

            # ---------------- stage A (row-packed 2x) ----------------
            with tc.tile_pool(name="xs", bufs=1) as xpool, \
                 tc.tile_pool(name="ys", bufs=1) as ypool, \
                 tc.tile_pool(name="psA", bufs=2, space="PSUM") as psA:
                x_s = xpool.tile([128, LD // 2], bf16, tag="xs")
                nc.sync.dma_start(x_s[0:64, :], x_p[:, 0:LD // 2])
                nc.sync.dma_start(x_s[64:128, :], x_p[:, LD // 2:LD])
                Y_s = ypool.tile([66, LD], bf16, tag="Ys")
                safe_barrier(tc, nc)
                for c in range(48):
                    sl = slice(512 * c, 512 * (c + 1))
                    sl2 = slice(LD // 2 + 512 * c, LD // 2 + 512 * (c + 1))
                    ps0 = psA.tile([66, 512], f32, tag="ps0")
                    ps1 = psA.tile([66, 512], f32, tag="ps1")
                    nc.tensor.matmul(ps0[:, :], cA2[0:64, 0:66], x_s[0:64, sl],
                                     start=True, stop=True)
                    nc.tensor.matmul(ps1[:, :], cA2[64:128, 66:132], x_s[64:128, sl],
                                     start=True, stop=True)
                    nc.scalar.activation(Y_s[:, sl], ps0[:, :], COPY)
                    nc.vector.tensor_copy(Y_s[:, sl2], ps1[:, :])
                safe_barrier(tc, nc)
                nc.sync.dma_start(Y_dram[:, :], Y_s[:, :])

            # ---------------- T1 + stage B ----------------
            with tc.tile_pool(name="X", bufs=1) as Xpool:
                X_t = [Xpool.tile([BS, NS * 128], bf16, tag=f"X{nb}", name=f"X{nb}")
                       for nb in range(NB)]
                with tc.tile_pool(name="yt", bufs=1) as ytpool, \
                     tc.tile_pool(name="psB", bufs=4, space="PSUM") as psB:
                    Y_T = ytpool.tile([128, NS * D], bf16, tag="YT")
                    nc.sync.dma_start(
                        Y_T[0:64, :].rearrange("p (s d) -> p s d", s=NS),
                        Y_dram[0:NS, :].rearrange("s (l d) -> l s d", l=G))
                    nc.sync.dma_start(
                        Y_T[64:128, :].rearrange("p (s d) -> p s d", s=NS),
                        Y_dram[NS:66, :].rearrange("s (l d) -> l s d", l=G))
                    for g in range(9):
                        ss = list(range(4 * g, min(4 * g + 4, NS)))
                        for nb in range(NB):
                            ps = psB.tile([BS, 512], f32, tag="psB")
                            for si, s in enumerate(ss):
                                nc.tensor.matmul(
                                    ps[:, si * 128:(si + 1) * 128],
                                    Y_T[:, s * D + nb * BS: s * D + nb * BS + BS],
                                    cB_s[:, s * 128:(s + 1) * 128],
                                    start=(si == 0), stop=(si == len(ss) - 1))
                            w_ = len(ss) * 128
                            dst = X_t[nb][:, 512 * g: 512 * g + w_]
                            if (g + nb) % 2 == 0:
                                nc.scalar.activation(dst, ps[:, 0:w_], COPY)
                            else:
                                nc.vector.tensor_copy(dst, ps[:, 0:w_])

                # ---------------- MLP ----------------
                safe_barrier(tc, nc)
                CH = [(0, 8), (8, 8), (16, 8), (24, 8), (32, 1)]  # (s0, n_s)
                with tc.tile_pool(name="o1", bufs=2) as o1pool, \
                     tc.tile_pool(name="o2", bufs=8) as o2pool, \
                     tc.tile_pool(name="sh", bufs=4) as shpool, \
                     tc.tile_pool(name="psM", bufs=2, space="PSUM") as psM:
                    for nb in range(NB):
                        X4 = X_t[nb][:, :].rearrange("p (s h k) -> p s h k", h=2, k=64)
                        w1r = w1_s[:, nb * 288: nb * 288 + 96]
                        w1i = w1_s[:, nb * 288 + 96: nb * 288 + 192]
                        w1ni = w1_s[:, nb * 288 + 192: nb * 288 + 288]
                        o1r = o1pool.tile([BS, NS * 64], bf16, tag="o1r")
                        o1i = o1pool.tile([BS, NS * 64], bf16, tag="o1i")
                        for s0, nsg in CH:
                            n = nsg * 64
                            rr = X4[:, s0:s0 + nsg, 0, :]
                            ri = X4[:, s0:s0 + nsg, 1, :]
                            pr = psM.tile([BS, 512], f32, tag="psMr")
                            pi = psM.tile([BS, 512], f32, tag="psMi")
                            nc.tensor.matmul(pr[:, 0:n], w1r, rr, start=True, stop=False)
                            nc.tensor.matmul(pi[:, 0:n], w1r, ri, start=True, stop=False)
                            nc.tensor.matmul(pr[:, 0:n], w1ni, ri, start=False, stop=True)
                            nc.tensor.matmul(pi[:, 0:n], w1i, rr, start=False, stop=True)
                            nc.scalar.activation(o1r[:, s0 * 64: s0 * 64 + n], pr[:, 0:n],
                                                 RELU, bias=b1_s[:, 2 * nb: 2 * nb + 1])
                            nc.scalar.activation(o1i[:, s0 * 64: s0 * 64 + n], pi[:, 0:n],
                                                 RELU, bias=b1_s[:, 2 * nb + 1: 2 * nb + 2])
                        w2r = w2_s[:, nb * 288: nb * 288 + 96]
                        w2i = w2_s[:, nb * 288 + 96: nb * 288 + 192]
                        w2ni = w2_s[:, nb * 288 + 192: nb * 288 + 288]
                        o2int = o2pool.tile([BS, NS * 128], bf16, tag="o2int")
                        o2v = o2int[:, :].rearrange("p (s x k) -> p s x k", x=2, k=64)
                        for s0, nsg in CH:
                            n = nsg * 64
                            c0 = s0 * 64
                            rr = o1r[:, c0:c0 + n]
                            ri = o1i[:, c0:c0 + n]
                            pr = psM.tile([BS, 512], f32, tag="ps2r")
                            pi = psM.tile([BS, 512], f32, tag="ps2i")
                            nc.tensor.matmul(pr[:, 0:n], w2r, rr, start=True, stop=False)
                            nc.tensor.matmul(pi[:, 0:n], w2r, ri, start=True, stop=False)
                            nc.tensor.matmul(pr[:, 0:n], w2ni, ri, start=False, stop=True)
                            nc.tensor.matmul(pi[:, 0:n], w2i, rr, start=False, stop=True)
                            for (ri_, psrc) in ((0, pr), (1, pi)):
                                bA = b2_s[:, 4 * nb + 2 * ri_: 4 * nb + 2 * ri_ + 1]
                                bC = b2_s[:, 4 * nb + 2 * ri_ + 1: 4 * nb + 2 * ri_ + 2]
                                s1 = shpool.tile([BS, 512], bf16, tag="s1")
                                s2 = shpool.tile([BS, 512], bf16, tag="s2")
                                nc.scalar.activation(s1[:, 0:n], psrc[:, 0:n], RELU,
                                                     bias=bA, scale=1.0)
                                nc.scalar.activation(s2[:, 0:n], psrc[:, 0:n], RELU,
                                                     bias=bC, scale=-1.0)
                                s1v = s1[:, 0:n].rearrange("p (s k) -> p s k", k=64)
                                s2v = s2[:, 0:n].rearrange("p (s k) -> p s k", k=64)
                                nc.vector.tensor_sub(o2v[:, s0:s0 + nsg, ri_, :],
                                                     s1v, s2v)
                        nc.sync.dma_start(o2_dram[nb], o2int[:, :])

            # ---------------- T2 + stage B' ----------------
            safe_barrier(tc, nc)
            with tc.tile_pool(name="g2", bufs=1) as g2pool:
                G2_s = g2pool.tile([128, NS * D], bf16, tag="G2")
                with tc.tile_pool(name="xp", bufs=1) as xppool, \
                     tc.tile_pool(name="psI", bufs=4, space="PSUM") as psI:
                    Xp = xppool.tile([128, NS * D], bf16, tag="Xp")
                    for nb in range(NB):
                        for s in range(NS):
                            nc.sync.dma_start(
                                Xp[:, s * D + nb * BS: s * D + (nb + 1) * BS],
                                o2_dram[nb][:, s * 128:(s + 1) * 128],
                                transpose=True)
                    safe_barrier(tc, nc)
                    for s in range(NS):
                        pa = psI.tile([128, 384], f32, tag="pIa")
                        pb = psI.tile([128, 384], f32, tag="pIb")
                        lhsT = cD_s[:, s * 128:(s + 1) * 128]
                        for nb in range(NB):
                            tgt = pa if nb < 4 else pb
                            col = (nb % 4) * BS
                            nc.tensor.matmul(
                                tgt[:, col:col + BS], lhsT,
                                Xp[:, s * D + nb * BS: s * D + (nb + 1) * BS],
                                start=(nb % 4 == 0), stop=(nb % 4 == 3))
                        nc.scalar.activation(G2_s[:, s * D: s * D + 384], pa[:, :], COPY)
                        nc.vector.tensor_copy(G2_s[:, s * D + 384: s * D + 768], pb[:, :])
                safe_barrier(tc, nc)

                # ---------------- T3 + stage A' (G2_s must stay live) --------
                with tc.tile_pool(name="gt", bufs=1) as gtpool, \
                     tc.tile_pool(name="psO", bufs=2, space="PSUM") as psO, \
                     tc.tile_pool(name="stO", bufs=3) as stO:
                    G2_T = gtpool.tile([66, LD], bf16, tag="GT")
                    for s in range(NS):
                        nc.sync.dma_start(
                            G2_T[s:s + 1, :].rearrange("p (l d) -> p l d", l=G),
                            G2_s[0:64, s * D:(s + 1) * D].rearrange(
                                "p (a d) -> p a d", a=1))
                        nc.sync.dma_start(
                            G2_T[NS + s:NS + s + 1, :].rearrange(
                                "p (l d) -> p l d", l=G),
                            G2_s[64:128, s * D:(s + 1) * D].rearrange(
                                "p (a d) -> p a d", a=1))
                    for c in range(96):
                        sl = slice(512 * c, 512 * (c + 1))
                        ps = psO.tile([G, 512], f32, tag="psO")
                        nc.tensor.matmul(ps[:, :], cAm_s[:, :], G2_T[:, sl],
                                         start=True, stop=True)
                        st = stO.tile([G, 512], bf16, tag="stO")
                        if c % 2 == 0:
                            nc.scalar.activation(st[:, :], ps[:, :], COPY)
                        else:
                            nc.vector.tensor_copy(st[:, :], ps[:, :])
                        nc.sync.dma_start(out_p[:, sl], st[:, :])
    _split_excess_waits(nc)
    return nc


def _get_graph():
    if "nc" not in _CACHE:
        _CACHE["nc"] = _build_graph()
    return _CACHE["nc"]


# ---------------------------------------------------------------- host entry
def kernel(x, w1, b1, w2, b2):
    nc = _get_graph()
    from concourse.bass_utils import run_bass_kernel_spmd

    cA, cB, cD, cAm = _build_matrices()
    cB_h = np.ascontiguousarray(cB.transpose(1, 0, 2)).reshape(128, NS * 128)
    cD_h = np.ascontiguousarray(cD.transpose(1, 0, 2)).reshape(128, NS * 128)
    w1_h = np.concatenate(
        [np.concatenate([w1[0, nb], w1[1, nb], -w1[1, nb]], axis=1) for nb in range(NB)],
        axis=1)                                            # [96, 8*288] (rows=i, cols=(nb,t,o))
    w2_h = np.concatenate(
        [np.concatenate([w2[0, nb], w2[1, nb], -w2[1, nb]], axis=1) for nb in range(NB)],
        axis=1)
    b1_h = np.stack([b1[ri, nb] for nb in range(NB) for ri in range(2)], axis=1)  # [96, 16]
    b2_h = np.stack(
        [v for nb in range(NB) for ri in range(2)
         for v in (b2[ri, nb] - LAM, -b2[ri, nb] - LAM)], axis=1)                 # [96, 32]

    consts = {
        "cA": cA.astype(BF), "cB": cB_h.astype(BF), "cD": cD_h.astype(BF),
        "cAm": cAm.astype(BF), "w1s": w1_h.astype(BF), "w2s": w2_h.astype(BF),
        "b1s": b1_h.astype(np.float32), "b2s": b2_h.astype(np.float32),
    }
    B = x.shape[0]
    in_maps = [dict(consts, x=np.ascontiguousarray(x[b].reshape(G, LD)).astype(BF))
               for b in range(B)]
    res = run_bass_kernel_spmd(nc, in_maps, core_ids=list(range(B)))
    _CACHE["last_result"] = res
    _CACHE["last_in_maps"] = in_maps
    y = np.empty_like(x)
    for b in range(B):
        delta = np.asarray(res.results[b]["delta"]).astype(np.float32).reshape(L, D)
        y[b] = x[b] + delta
    return y


# revision 19
# speedup vs baseline: 1.2475x; 1.2468x over previous
"""AdaptiveFNOFilter1d Trainium2 kernel.

Per-sample pipeline (8 samples -> 8 NeuronCores, pure data parallel):
  rfft4096 (Cooley-Tukey 64x64 as TensorE matmuls) -> complex block-diag MLP
  (relu, softshrink) -> irfft4096 -> +x residual (on host, in fp32).

All device compute in bf16 (fp32 PSUM accumulation). Mode 2048 dropped
(contributes ~1e-4 relative; validated 6e-4 total rel err vs reference).

Layouts (l = l1 + 64*l2, k = k2 + 64*k1, modes stored for k2-slices s=0..32):
  stage A  : Y[k2ri 66, (l1 64, d 768)] = cA.T @ x[l2 64, (l1 d)]   (row-packed 2x)
  T1 (DRAM): -> Y_T[l1ri 128, (s 33, d 768)]
  stage B  : per (s, d-block 96): X[d96, 128] = Y_T-slice.T @ cB[s]
             X cols per s: [dr 32 | mr 32 | di 32 | mi 32] (mr/mi = modes k2=64-s)
  MLP      : channels on partitions, modes on free dim; ACT fuses bias+relu+evac;
             softshrink = relu(v-lam) - relu(-v-lam) as two ACT passes + DVE sub
  T2 (DRAM): -> Xp[128 rows dr/mr/di/mi, (s 33, d 768)]
  stage B' : per s: G2[l1ri 128, 768] = cD[s].T @ Xp-slice
  T3 (SBUF): -> G2_T[k2ri 66, (l1 64, d 768)]  (direct SBUF->SBUF slice DMAs)
  stage A' : delta[l2 64, (l1 d)] = cAm.T @ G2_T
"""
import numpy as np
import ml_dtypes

L, G, D, NB, BS = 4096, 64, 768, 8, 96
LAM = 0.01
NS = 33
LD = G * D            # 49152
BF = ml_dtypes.bfloat16

_CACHE = {}


# ---------------------------------------------------------------- matrices
def _build_matrices():
    j = np.arange(G)
    ang = 2 * np.pi * np.outer(j, j) / G
    C64, S64 = np.cos(ang), np.sin(ang)
    cA = np.concatenate([C64[:, :NS] / 64.0, -S64[:, :NS] / 64.0], axis=1)

    l1 = np.arange(G)[:, None]
    k1 = np.arange(32)[None, :]

    def bmat(k2):
        th = 2 * np.pi * (k2 + 64.0 * k1) * l1 / L
        c, s = np.cos(th), np.sin(th)
        M = np.zeros((128, 64))
        M[0:64, 0:32] = c
        M[64:128, 0:32] = s
        M[0:64, 32:64] = -s
        M[64:128, 32:64] = c
        return M

    cB = np.zeros((NS, 128, 128))
    for s in range(NS):
        direct = bmat(s)
        cB[s, :, 0:32] = direct[:, 0:32]
        cB[s, :, 64:96] = direct[:, 32:64]
        if 0 < s < 32:
            mir = bmat(64 - s)
            mir[64:128, :] *= -1.0
            cB[s, :, 32:64] = mir[:, 0:32]
            cB[s, :, 96:128] = mir[:, 32:64]

    l1r = np.arange(G)[None, :]
    k1c = np.arange(32)[:, None]
    cD = np.zeros((NS, 128, 128))
    for s in range(NS):
        th = 2 * np.pi * (s + 64.0 * k1c) * l1r / L
        c, s_ = np.cos(th), np.sin(th)
        dir_r = np.concatenate([c, s_], axis=1)
        dir_i = np.concatenate([-s_, c], axis=1)
        if 0 < s < 32:
            thm = 2 * np.pi * (s + 64.0 * (63 - k1c)) * l1r / L
        elif s == 0:
            thm = 2 * np.pi * (64.0 * ((64 - k1c) % 64)) * l1r / L
        else:
            thm = 2 * np.pi * (32 + 64.0 * (63 - k1c)) * l1r / L
        cm, sm = np.cos(thm), np.sin(thm)
        mir_r = np.concatenate([cm, sm], axis=1)
        mir_i = np.concatenate([sm, -cm], axis=1)
        if s == 0:
            mir_r[0, :] = 0.0
            mir_i[0, :] = 0.0
        if s in (0, 32):
            cD[s, 0:32] = dir_r + mir_r
            cD[s, 64:96] = dir_i + mir_i
        else:
            cD[s, 0:32] = dir_r
            cD[s, 32:64] = mir_r
            cD[s, 64:96] = dir_i
            cD[s, 96:128] = mir_i

    w = np.full(NS, 2.0)
    w[0] = 1.0
    w[32] = 1.0
    cAm = np.concatenate([(w[:, None] * C64[:NS, :]) / 64.0,
                          (w[:, None] * -S64[:NS, :]) / 64.0], axis=0)
    return cA, cB, cD, cAm


# ---------------------------------------------------------------- graph
def _build_graph():
    import concourse.bass as bass
    import concourse.mybir as mybir
    import concourse.tile as tile

    f32 = mybir.dt.float32
    bf16 = mybir.dt.bfloat16
    RELU = mybir.ActivationFunctionType.Relu
    COPY = mybir.ActivationFunctionType.Copy

    nc = bass.Bass()
    x_p = nc.declare_dram_parameter("x", [G, LD], bf16, isOutput=False)
    cA_p = nc.declare_dram_parameter("cA", [G, 66], bf16, isOutput=False)
    cB_p = nc.declare_dram_parameter("cB", [128, NS * 128], bf16, isOutput=False)
    cD_p = nc.declare_dram_parameter("cD", [128, NS * 128], bf16, isOutput=False)
    cAm_p = nc.declare_dram_parameter("cAm", [66, G], bf16, isOutput=False)
    w1_p = nc.declare_dram_parameter("w1s", [BS, NB * 3 * BS], bf16, isOutput=False)
    w2_p = nc.declare_dram_parameter("w2s", [BS, NB * 3 * BS], bf16, isOutput=False)
    b1_p = nc.declare_dram_parameter("b1s", [BS, NB * 2], f32, isOutput=False)
    b2_p = nc.declare_dram_parameter("b2s", [BS, NB * 4], f32, isOutput=False)
    out_p = nc.declare_dram_parameter("delta", [G, LD], bf16, isOutput=True)

    Y_dram = nc.dram_tensor("Y_dram", [66, LD], bf16)
    # o2 spectrum, interleaved per s: cols = (s 33, ri 2, kk 64)
    o2_dram = nc.dram_tensor("o2_dram", [NB, BS, NS * 128], bf16)


    from concourse.tile import add_dep_helper

    def safe_barrier(tc, nc):
        """All-engine barrier that never puts >2 sync waits on one instruction:
        a chain of sync nops each absorbing one producer, then installed as the
        block barrier so every later instruction deps only on the final nop."""
        curr_bb = nc.cur_bb
        prev = list(curr_bb.bb.instructions)
        last_by_engine = {}
        dmas = []
        for i in prev:
            if not i.is_executable():
                continue
            last_by_engine[str(i.engine)] = i
            if "Dma" in type(i).__name__ or "DMA" in type(i).__name__:
                dmas.append(i)
        targets = [v for v in last_by_engine.values()]
        for d in dmas[-8:]:
            if all(d is not t for t in targets):
                targets.append(d)
        n = None
        for t in targets:
            n = nc.sync.nop()
            add_dep_helper(
                n.ins, t,
                sync=bass.sync_unless_reorderable_target(t, t.is_executable()),
                reason="safe_barrier")
        if n is not None:
            tc.barrier_instruction_and_bb = (n.ins, curr_bb)
            if (tc.no_sync_barrier_and_bb is not None
                    and tc.no_sync_barrier_and_bb[1] == curr_bb):
                tc.no_sync_barrier_and_bb = None

    def _split_excess_waits(nc, max_attached=1):
        """Walrus accepts ~1 sync-wait per instruction. Hoist extras onto
        standalone same-engine NoOps inserted immediately before (the raw-bass
        wait_ge idiom), preserving per-engine program order."""
        wid = [0]
        for f in nc.m.functions:
            new_blocks = []
            changed = False
            for b in f.blocks:
                insts = list(b.instructions)
                if not any(i.sync_info and len(i.sync_info.on_wait) > max_attached
                           for i in insts):
                    new_blocks.append(b)
                    continue
                changed = True
                out = []
                for i in insts:
                    si = i.sync_info
                    if si and len(si.on_wait) > max_attached:
                        waits = list(si.on_wait)
                        for w in waits[:-max_attached]:
                            k = mybir.InstNoOp(name=f"I-wsp{wid[0]}", ins=[], outs=[])
                            wid[0] += 1
                            k.engine = i.engine
                            k.sync_info = mybir.SyncInfo(on_wait=[w], on_update=[])
                            out.append(k)
                        i.sync_info = mybir.SyncInfo(
                            on_wait=waits[-max_attached:],
                            on_update=list(si.on_update))
                    out.append(i)
                nb = type(b)(name=b.name, instructions=out)
                nb.IsExit = b.IsExit
                nb.IsLoopEntry = b.IsLoopEntry
                nb.IsPredicated = b.IsPredicated
                new_blocks.append(nb)
            if changed:
                f.blocks = new_blocks

    with tile.TileContext(nc) as tc:
        with tc.tile_pool(name="const", bufs=1) as cpool:
            cA2 = cpool.tile([128, 132], bf16, tag="cA2")
            nc.sync.dma_start(cA2[0:64, 0:66], cA_p[:, :])
            nc.sync.dma_start(cA2[64:128, 66:132], cA_p[:, :])
            cB_s = cpool.tile([128, NS * 128], bf16, tag="cB")
            nc.sync.dma_start(cB_s[:, :], cB_p[:, :])
            cD_s = cpool.tile([128, NS * 128], bf16, tag="cD")
            nc.sync.dma_start(cD_s[:, :], cD_p[:, :])
            cAm_s = cpool.tile([66, G], bf16, tag="cAm")
            nc.sync.dma_start(cAm_s[:, :], cAm_p[:, :])
            w1_s = cpool.tile([BS, NB * 3 * BS], bf16, tag="w1")
            nc.sync.dma_start(w1_s[:, :], w1_p[:, :])
            w2_s = cpool.tile([BS, NB * 3 * BS], bf16, tag="w2")
            nc.sync.dma_start(w2_s[:, :], w2_p[:, :])
            b1_s = cpool.tile([BS, NB * 2], f32, tag="b1")
            nc.sync.dma_start(b1_s[:, :], b1_p[:, :])
            b2_s = cpool.tile([BS, NB * 4], f32, tag="b2")
            nc.sync.dma_start(b2_s[:, :], b2_p[:, :])

            # ---------------- stage A (row-packed 2x) ----------------
            with tc.tile_pool(name="xs", bufs=1) as xpool, \
                 tc.tile_pool(name="ys", bufs=1) as ypool, \
                 tc.tile_pool(name="psA", bufs=2, space="PSUM") as psA:
                x_s = xpool.tile([128, LD // 2], bf16, tag="xs")
                nc.sync.dma_start(x_s[0:64, :], x_p[:, 0:LD // 2])
                nc.sync.dma_start(x_s[64:128, :], x_p[:, LD // 2:LD])
                Y_s = ypool.tile([66, LD], bf16, tag="Ys")
                safe_barrier(tc, nc)
                for c in range(48):
                    sl = slice(512 * c, 512 * (c + 1))
                    sl2 = slice(LD // 2 + 512 * c, LD // 2 + 512 * (c + 1))
                    ps0 = psA.tile([66, 512], f32, tag="ps0")
                    ps1 = psA.tile([66, 512], f32, tag="ps1")
                    nc.tensor.matmul(ps0[:, :], cA2[0:64, 0:66], x_s[0:64, sl],
                                     start=True, stop=True)
                    nc.tensor.matmul(ps1[:, :], cA2[64:128, 66:132], x_s[64:128, sl],
                                     start=True, stop=True)
                    nc.scalar.activation(Y_s[:, sl], ps0[:, :], COPY)
                    nc.vector.tensor_copy(Y_s[:, sl2], ps1[:, :])
                safe_barrier(tc, nc)
                nc.sync.dma_start(Y_dram[:, :], Y_s[:, :])

            # ---------------- T1 + stage B ----------------
            with tc.tile_pool(name="X", bufs=1) as Xpool:
                X_t = [Xpool.tile([BS, NS * 128], bf16, tag=f"X{nb}", name=f"X{nb}")
                       for nb in range(NB)]
                with tc.tile_pool(name="yt", bufs=1) as ytpool, \
                     tc.tile_pool(name="psB", bufs=4, space="PSUM") as psB:
                    Y_T = ytpool.tile([128, NS * D], bf16, tag="YT")
                    nc.sync.dma_start(
                        Y_T[0:64, :].rearrange("p (s d) -> p s d", s=NS),
                        Y_dram[0:NS, :].rearrange("s (l d) -> l s d", l=G))
                    nc.sync.dma_start(
                        Y_T[64:128, :].rearrange("p (s d) -> p s d", s=NS),
                        Y_dram[NS:66, :].rearrange("s (l d) -> l s d", l=G))
                    for g in range(9):
                        ss = list(range(4 * g, min(4 * g + 4, NS)))
                        for nb in range(NB):
                            ps = psB.tile([BS, 512], f32, tag="psB")
                            for si, s in enumerate(ss):
                                nc.tensor.matmul(
                                    ps[:, si * 128:(si + 1) * 128],
                                    Y_T[:, s * D + nb * BS: s * D + nb * BS + BS],
                                    cB_s[:, s * 128:(s + 1) * 128],
                                    start=(si == 0), stop=(si == len(ss) - 1))
                            w_ = len(ss) * 128
                            dst = X_t[nb][:, 512 * g: 512 * g + w_]
                            if (g + nb) % 2 == 0:
                                nc.scalar.activation(dst, ps[:, 0:w_], COPY)
                            else:
                                nc.vector.tensor_copy(dst, ps[:, 0:w_])

                # ---------------- MLP ----------------
                safe_barrier(tc, nc)
                CH = [(0, 8), (8, 8), (16, 8), (24, 8), (32, 1)]  # (s0, n_s)
                with tc.tile_pool(name="o1", bufs=2) as o1pool, \
                     tc.tile_pool(name="o2", bufs=8) as o2pool, \
                     tc.tile_pool(name="sh", bufs=4) as shpool, \
                     tc.tile_pool(name="psM", bufs=2, space="PSUM") as psM:
                    for nb in range(NB):
                        X4 = X_t[nb][:, :].rearrange("p (s h k) -> p s h k", h=2, k=64)
                        w1r = w1_s[:, nb * 288: nb * 288 + 96]
                        w1i = w1_s[:, nb * 288 + 96: nb * 288 + 192]
                        w1ni = w1_s[:, nb * 288 + 192: nb * 288 + 288]
                        o1r = o1pool.tile([BS, NS * 64], bf16, tag="o1r")
                        o1i = o1pool.tile([BS, NS * 64], bf16, tag="o1i")
                        for s0, nsg in CH:
                            n = nsg * 64
                            rr = X4[:, s0:s0 + nsg, 0, :]
                            ri = X4[:, s0:s0 + nsg, 1, :]
                            pr = psM.tile([BS, 512], f32, tag="psMr")
                            pi = psM.tile([BS, 512], f32, tag="psMi")
                            nc.tensor.matmul(pr[:, 0:n], w1r, rr, start=True, stop=False)
                            nc.tensor.matmul(pi[:, 0:n], w1r, ri, start=True, stop=False)
                            nc.tensor.matmul(pr[:, 0:n], w1ni, ri, start=False, stop=True)
                            nc.tensor.matmul(pi[:, 0:n], w1i, rr, start=False, stop=True)
                            nc.scalar.activation(o1r[:, s0 * 64: s0 * 64 + n], pr[:, 0:n],
                                                 RELU, bias=b1_s[:, 2 * nb: 2 * nb + 1])
                            nc.scalar.activation(o1i[:, s0 * 64: s0 * 64 + n], pi[:, 0:n],
                                                 RELU, bias=b1_s[:, 2 * nb + 1: 2 * nb + 2])
                        w2r = w2_s[:, nb * 288: nb * 288 + 96]
                        w2i = w2_s[:, nb * 288 + 96: nb * 288 + 192]
                        w2ni = w2_s[:, nb * 288 + 192: nb * 288 + 288]
                        o2int = o2pool.tile([BS, NS * 128], bf16, tag="o2int")
                        o2v = o2int[:, :].rearrange("p (s x k) -> p s x k", x=2, k=64)
                        for s0, nsg in CH:
                            n = nsg * 64
                            c0 = s0 * 64
                            rr = o1r[:, c0:c0 + n]
                            ri = o1i[:, c0:c0 + n]
                            pr = psM.tile([BS, 512], f32, tag="ps2r")
                            pi = psM.tile([BS, 512], f32, tag="ps2i")
                            nc.tensor.matmul(pr[:, 0:n], w2r, rr, start=True, stop=False)
                            nc.tensor.matmul(pi[:, 0:n], w2r, ri, start=True, stop=False)
                            nc.tensor.matmul(pr[:, 0:n], w2ni, ri, start=False, stop=True)
                            nc.tensor.matmul(pi[:, 0:n], w2i, rr, start=False, stop=True)
                            for (ri_, psrc) in ((0, pr), (1, pi)):
                                bA = b2_s[:, 4 * nb + 2 * ri_: 4 * nb + 2 * ri_ + 1]
                                bC = b2_s[:, 4 * nb + 2 * ri_ + 1: 4 * nb + 2 * ri_ + 2]
                                s1 = shpool.tile([BS, 512], bf16, tag="s1")
                                s2 = shpool.tile([BS, 512], bf16, tag="s2")
                                nc.scalar.activation(s1[:, 0:n], psrc[:, 0:n], RELU,
                                                     bias=bA, scale=1.0)
                                nc.scalar.activation(s2[:, 0:n], psrc[:, 0:n], RELU,
                                                     bias=bC, scale=-1.0)
                                s1v = s1[:, 0:n].rearrange("p (s k) -> p s k", k=64)
                                s2v = s2[:, 0:n].rearrange("p (s k) -> p s k", k=64)
                                nc.vector.tensor_sub(o2v[:, s0:s0 + nsg, ri_, :],
                                                     s1v, s2v)
                        nc.sync.dma_start(o2_dram[nb], o2int[:, :])

            # ---------------- T2 + stage B' ----------------
            safe_barrier(tc, nc)
            with tc.tile_pool(name="g2", bufs=1) as g2pool:
                G2_s = g2pool.tile([128, NS * D], bf16, tag="G2")
                with tc.tile_pool(name="xp", bufs=1) as xppool, \
                     tc.tile_pool(name="psI", bufs=4, space="PSUM") as psI:
                    Xp = xppool.tile([128, NS * D], bf16, tag="Xp")
                    for nb in range(NB):
                        for s in range(NS):
                            nc.sync.dma_start(
                                Xp[:, s * D + nb * BS: s * D + (nb + 1) * BS],
                                o2_dram[nb][:, s * 128:(s + 1) * 128],
                                transpose=True)
                    safe_barrier(tc, nc)
                    for s in range(NS):
                        pa = psI.tile([128, 384], f32, tag="pIa")
                        pb = psI.tile([128, 384], f32, tag="pIb")
                        lhsT = cD_s[:, s * 128:(s + 1) * 128]
                        for nb in range(NB):
                            tgt = pa if nb < 4 else pb
                            col = (nb % 4) * BS
                            nc.tensor.matmul(
                                tgt[:, col:col + BS], lhsT,
                                Xp[:, s * D + nb * BS: s * D + (nb + 1) * BS],
                                start=(nb % 4 == 0), stop=(nb % 4 == 3))
                        nc.scalar.activation(G2_s[:, s * D: s * D + 384], pa[:, :], COPY)
                        nc.vector.tensor_copy(G2_s[:, s * D + 384: s * D + 768], pb[:, :])
                safe_barrier(tc, nc)

                # ---------------- T3 + stage A' (G2_s must stay live) --------
                with tc.tile_pool(name="gt", bufs=1) as gtpool, \
                     tc.tile_pool(name="psO", bufs=2, space="PSUM") as psO, \
                     tc.tile_pool(name="stO", bufs=3) as stO:
                    G2_T = gtpool.tile([66, LD], bf16, tag="GT")
                    for s in range(NS):
                        nc.sync.dma_start(
                            G2_T[s:s + 1, :].rearrange("p (l d) -> p l d", l=G),
                            G2_s[0:64, s * D:(s + 1) * D].rearrange(
                                "p (a d) -> p a d", a=1))
                        nc.sync.dma_start(
                            G2_T[NS + s:NS + s + 1, :].rearrange(
                                "p (l d) -> p l d", l=G),
                            G2_s[64:128, s * D:(s + 1) * D].rearrange(
                                "p (a d) -> p a d", a=1))
                    for c in range(96):
                        sl = slice(512 * c, 512 * (c + 1))
                        ps = psO.tile([G, 512], f32, tag="psO")
                        nc.tensor.matmul(ps[:, :], cAm_s[:, :], G2_T[:, sl],
                                         start=True, stop=True)
                        st = stO.tile([G, 512], bf16, tag="stO")
                        if c % 2 == 0:
                            nc.scalar.activation(st[:, :], ps[:, :], COPY)
                        else:
                            nc.vector.tensor_copy(st[:, :], ps[:, :])
                        nc.sync.dma_start(out_p[:, sl], st[:, :])
    _split_excess_waits(nc)
    return nc


def _get_graph():
    if "nc" not in _CACHE:
        _CACHE["nc"] = _build_graph()
    return _CACHE["nc"]


# ---------------------------------------------------------------- host entry
def kernel(x, w1, b1, w2, b2):
    nc = _get_graph()
    from concourse.bass_utils import run_bass_kernel_spmd

    cA, cB, cD, cAm = _build_matrices()
    cB_h = np.ascontiguousarray(cB.transpose(1, 0, 2)).reshape(128, NS * 128)
    cD_h = np.ascontiguousarray(cD.transpose(1, 0, 2)).reshape(128, NS * 128)
    w1_h = np.concatenate(
        [np.concatenate([w1[0, nb], w1[1, nb], -w1[1, nb]], axis=1) for nb in range(NB)],
        axis=1)                                            # [96, 8*288] (rows=i, cols=(nb,t,o))
    w2_h = np.concatenate(
        [np.concatenate([w2[0, nb], w2[1, nb], -w2[1, nb]], axis=1) for nb in range(NB)],
        axis=1)
    b1_h = np.stack([b1[ri, nb] for nb in range(NB) for ri in range(2)], axis=1)  # [96, 16]
    b2_h = np.stack(
        [v for nb in range(NB) for ri in range(2)
         for v in (b2[ri, nb] - LAM, -b2[ri, nb] - LAM)], axis=1)                 # [96, 32]

    consts = {
        "cA": cA.astype(BF), "cB": cB_h.astype(BF), "cD": cD_h.astype(BF),
        "cAm": cAm.astype(BF), "w1s": w1_h.astype(BF), "w2s": w2_h.astype(BF),
        "b1s": b1_h.astype(np.float32), "b2s": b2_h.astype(np.float32),
    }
    B = x.shape[0]
    in_maps = [dict(consts, x=np.ascontiguousarray(x[b].reshape(G, LD)).astype(BF))
               for b in range(B)]
    res = run_bass_kernel_spmd(nc, in_maps, core_ids=list(range(B)))
    _CACHE["last_result"] = res
    _CACHE["last_in_maps"] = in_maps
    y = np.empty_like(x)
    for b in range(B):
        delta = np.asarray(res.results[b]["delta"]).astype(np.float32).reshape(L, D)
        y[b] = x[b] + delta
    return y
